# revision 1
# baseline (speedup 1.0000x reference)
"""Trainium2 Bass kernel for ChannelDepsModule (sequential channel recurrence).

Math (per pixel, fp32):
    m_0 = mix_0 ; ybar_0 = round(x_0 - m_0) + m_0
    for i in 1..191:
        m_i = sum_{c<i} Wm[i-1,c] * ybar_c + b[i-1] + mix_i
        ybar_i = round(x_i - m_i) + m_i
    outputs: ybar, mix_out (= m)

Device strategy (per core, one batch image, 4096 pixels):
  - pixels on SBUF partitions ([128] x 32 chunks), channels on the free dim
  - channels in 6 blocks of 32:
      * cross-block mix contributions P via TensorE matmuls
        (stationary ybar in channel-partition layout x Wm^T slice), with
        pixel-partition PSUM output; q = x - mix - b - P is written into the
        block's ybar columns ahead of time
      * in-block recurrence: one fused DVE scan per channel computes
        t_i = q_i - sum_j w_ij y_j directly (weights negated, +1 planted on
        the diagonal so the prefilled q column enters the dot), then one
        fused DVE op assembles y_i = round(t_i) + (x_i - t_i) using the
        +-1.5*2^23 magic constant (IEEE RNE == jnp.round)
      * mix_out column m_i = x_i - t_i is produced on the GpSimd engine,
        off the critical path
      * finished ybar columns are TensorE-transposed (two half-block waves)
        into channel-partition tiles for later blocks' matmuls
  - b is folded into mix on the host; mix_out channel 0 restored on host
"""

import sys

import numpy as np

if "/opt/trn_rl_repo" not in sys.path:
    sys.path.insert(0, "/opt/trn_rl_repo")

N, C, H, Wd = 8, 192, 64, 64
NPIX = H * Wd          # 4096 pixels per core
B = 32                 # channel block size
NBLK = C // B          # 6
ROUND_C = 1.5 * 2.0**23  # fp32 add of this rounds to nearest-even integer

_CACHE = {}
_DVE_OPS = {}


def _register_dve_ops():
    """Define + register the two fused DVE ops (idempotent)."""
    if _DVE_OPS:
        return _DVE_OPS
    import concourse.dve_ops as dops
    import concourse.dve_spec as ds
    from concourse.dve_spec import AluOp, Spec, Src0, Src1
    from concourse.dve_ops import CUSTOM_DVE_SPECS, OPS, DveOp
    from concourse.dve_uop import DveOpSpec

    # The stock segmented-scan machinery only implements the page-counter
    # mode; add the documented per-page *reset* behavior for scans marked
    # with `_page_reset`: at each SUB_DIM_DONE the STEP state computes
    # d <- op(init, expr) instead of op(CURR, expr).
    if not getattr(ds, "_page_reset_patched", False):
        _orig = ds._scan_overrides

        def _patched(scans, node_stage):
            seed, step = _orig(scans, node_stage)
            for sc in scans:
                if getattr(sc, "_page_reset", False):
                    d = node_stage[sc]
                    step[d] = ds._Stage(sc.op, ds._scan_init(sc), sc.expr)
            return seed, step

        ds._scan_overrides = _patched
        ds._page_reset_patched = True

    def _chaindot_ref(in0, in1, s0, s1, imm2):
        p = in0.shape[0]
        inner = in0.shape[-1]
        a = in0.reshape(p, -1, inner).astype(np.float32)
        bb = in1.reshape(p, -1, inner).astype(np.float32)
        return np.cumsum(a * bb, axis=-1, dtype=np.float32).reshape(in0.shape)

    sc = ds.scan(AluOp.ADD, Src0 * Src1)
    object.__setattr__(sc, "_page_reset", True)
    spec_cd = Spec(body=sc, reference=_chaindot_ref)

    def _quanty_ref(in0, in1, s0, s1, imm2):
        c = np.float32(s0)
        t = in0.astype(np.float32)
        return ((t + c) - c) + (in1.astype(np.float32) - t)

    spec_qy = Spec(
        body=((Src0 + ds.C0) - ds.C0) + (Src1 - Src0), reference=_quanty_ref
    )

    def _mk(name, spec, subdim):
        if any(o.name == name for o in OPS):
            op = next(o for o in OPS if o.name == name)
        else:
            shas = {}
            for ver in ("v3", "v4"):
                shas[ver] = DveOpSpec(
                    name=name, uops=ds.lower(spec, ver=ver)
                ).sha(ver)
            op = DveOp(name, spec, subdim=subdim, uops_sha=shas)
            OPS.append(op)
            CUSTOM_DVE_SPECS[name] = spec
            dops._SUB_OPCODE_FOR_NAME[name] = dops._CUSTOM_DVE_ROW_BASE + len(OPS) - 1
        return op

    _DVE_OPS["chaindot"] = _mk("CHAINDOT_SEQ_ANT", spec_cd, subdim=True)
    _DVE_OPS["quanty"] = _mk("QUANTY_ANT", spec_qy, subdim=False)
    return _DVE_OPS


def _build(n_chunks):
    """Build + compile the per-core Bass module. n_chunks pixel chunks of 128."""
    import concourse.bacc as bacc
    import concourse.mybir as mybir
    from concourse.tile import TileContext

    ops = _register_dve_ops()
    npix = n_chunks * 128
    fp32 = mybir.dt.float32

    nc = bacc.Bacc(None, target_bir_lowering=False)

    xt = nc.dram_tensor("xt", [npix, C], fp32, kind="ExternalInput")
    mixt = nc.dram_tensor("mixt", [npix, C], fp32, kind="ExternalInput")
    wt = nc.dram_tensor("wt", [C, C], fp32, kind="ExternalInput")
    wtri = nc.dram_tensor("wtri", [1, NBLK * B * B], fp32, kind="ExternalInput")
    ident = nc.dram_tensor("ident", [128, 128], fp32, kind="ExternalInput")
    yt = nc.dram_tensor("yt", [npix, C], fp32, kind="ExternalOutput")
    mot = nc.dram_tensor("mot", [npix, C], fp32, kind="ExternalOutput")

    K = n_chunks  # pixel chunks
    HB = B // 2   # transpose wave width

    with TileContext(nc) as tc:
        with (
            tc.tile_pool(name="big", bufs=1) as big,
            tc.tile_pool(name="small", bufs=1) as small,
            tc.tile_pool(name="scr", bufs=3) as scr,
            tc.tile_pool(name="qp", bufs=2) as qp,
            tc.tile_pool(name="psum_e", bufs=2, space="PSUM") as psum_e,
            tc.tile_pool(name="psum_f", bufs=1, space="PSUM") as psum_f,
            tc.tile_pool(name="psumt", bufs=2, space="PSUM") as psumt,
        ):
            # pixel-partition tiles, free layout = k*192 + c
            X = big.tile([128, K * C], fp32, tag="X")
            MIX = big.tile([128, K * C], fp32, tag="MIX")  # becomes mix_out
            XMB = big.tile([128, K * C], fp32, tag="XMB")
            Y = big.tile([128, K * C], fp32, tag="Y")
            # channel-partition decoded ybar: chans 0-127 / 128-159
            ysb_lo = big.tile([128, npix], fp32, tag="ysb_lo")
            ysb_hi = big.tile([32, npix], fp32, tag="ysb_hi")

            wt_lo = small.tile([128, C], fp32, tag="wt_lo")
            wt_hi = small.tile([64, C], fp32, tag="wt_hi")
            wtri_t = small.tile([1, NBLK * B * B], fp32, tag="wtri")
            wtri_b = small.tile([128, NBLK * B * B], fp32, tag="wtri_b")
            id_t = small.tile([128, 128], fp32, tag="ident")

            def big_in(tile, dram):
                nc.sync.dma_start(
                    tile[:].rearrange("p (k c) -> p k c", c=C),
                    dram[:].rearrange("(k p) c -> p k c", p=128),
                )

            big_in(X, xt)
            big_in(MIX, mixt)
            nc.sync.dma_start(wt_lo[:], wt[0:128, :])
            nc.sync.dma_start(wt_hi[:], wt[128:C, :])
            nc.sync.dma_start(wtri_t[:], wtri[:])
            nc.sync.dma_start(id_t[:], ident[:])
            nc.gpsimd.partition_broadcast(wtri_b[:], wtri_t[:])

            # XMB = X - (MIX + b)  (b folded into MIX on host)
            nc.vector.tensor_sub(XMB[:], X[:], MIX[:])

            def col(tile, ch):  # strided [128, K] view of channel ch
                return tile[:].rearrange("p (k c) -> p k c", c=C)[:, :, ch]

            def ycols(sb, j0, j1):  # [128, K, j1-j0] view of block sb's cols
                return (
                    Y[:]
                    .rearrange("p (k c) -> p k c", c=C)[
                        :, :, sb * B + j0 : sb * B + j1
                    ]
                )

            def xmb_slice(sb):
                return XMB[:].rearrange("p (k c) -> p k c", c=C)[
                    :, :, sb * B : (sb + 1) * B
                ]

            # PSUM accumulation-group bookkeeping: start=True marks the whole
            # 2KB bank pending-zero, so the first matmul touching each bank
            # opens the group (later writes to untouched bytes overwrite,
            # rewrites accumulate) and the last one per bank closes it.
            BANK_CHUNKS = 512 // B  # chunk-slices per PSUM bank

            def p_early(sb, pp):
                """Early P for block sb: chans [0, 32(sb-1)) — one matmul/chunk."""
                kdec = (sb - 1) * B
                for k in range(K):
                    nc.tensor.matmul(
                        pp[:, k * B : (k + 1) * B],
                        ysb_lo[0:kdec, k * 128 : (k + 1) * 128],
                        wt_lo[0:kdec, sb * B : (sb + 1) * B],
                        start=(k % BANK_CHUNKS == 0),
                        stop=False,
                    )

            def p_final(sb, pp, had_early):
                """Final P seg for block sb: block sb-1's 32 chans."""
                r0 = (sb - 1) * B
                ys, wtile, rr = (
                    (ysb_lo, wt_lo, r0) if r0 < 128 else (ysb_hi, wt_hi, r0 - 128)
                )
                tp = (rr, 0) if rr not in (0, 32, 64) else None
                for k in range(K):
                    nc.tensor.matmul(
                        pp[:, k * B : (k + 1) * B],
                        ys[rr : rr + B, k * 128 : (k + 1) * 128],
                        wtile[rr : rr + B, sb * B : (sb + 1) * B],
                        start=(not had_early) and (k % BANK_CHUNKS == 0),
                        stop=(k % BANK_CHUNKS == BANK_CHUNKS - 1) or (k == K - 1),
                        tile_position=tp,
                    )

            def q_fin(sb, pp):
                """Block sb's Y cols = XMB - PP."""
                nc.vector.tensor_sub(
                    ycols(sb, 0, B),
                    xmb_slice(sb),
                    pp[:].rearrange("p (k c) -> p k c", c=B),
                )

            def transpose_block(sb):
                """Transpose Y cols of block sb into ysb (chan-part)."""
                base = sb * B
                if base < 128:
                    dst, dr0 = ysb_lo, base
                else:
                    dst, dr0 = ysb_hi, base - 128
                for g in range(0, K, 4):
                    gn = min(4, K - g)
                    pt = psumt.tile([B, 512], fp32, tag="pt")
                    for t_i in range(gn):
                        k = g + t_i
                        nc.tensor.transpose(
                            pt[:, t_i * 128 : (t_i + 1) * 128],
                            Y[:, k * C + base : k * C + base + B],
                            id_t[:],
                        )
                    nc.scalar.copy(
                        dst[dr0 : dr0 + B, g * 128 : g * 128 + gn * 128],
                        pt[:, 0 : gn * 128],
                    )

            def steps(sb):
                base = sb * B
                for i in range(B):
                    ch = base + i
                    if i > 0:
                        prod = scr.tile([128, B * K], fp32, tag="prod")
                        pr = prod[:].rearrange("p (k c) -> p k c", c=B)[
                            :, :, 0 : i + 1
                        ]
                        woff = sb * B * B + i * B
                        wrow = (
                            wtri_b[:, woff : woff + i + 1]
                            .unsqueeze(1)
                            .broadcast_to([128, K, i + 1])
                        )
                        nc.vector._custom_dve(
                            ops["chaindot"], out=pr, in0=ycols(sb, 0, i + 1),
                            in1=wrow,
                        )
                        t_ap = prod[:].rearrange("p (k c) -> p k c", c=B)[:, :, i]
                    else:
                        t_ap = col(Y, ch)
                    # mix_out column (off critical path, on GpSimd)
                    nc.gpsimd.tensor_sub(col(MIX, ch), col(X, ch), t_ap)
                    # y = round(t) + (x - t)
                    nc.vector._custom_dve(
                        ops["quanty"], out=col(Y, ch), in0=t_ap,
                        in1=col(X, ch), s0=ROUND_C,
                    )

            # ---------------- schedule ----------------
            pp_cur = None
            for sb in range(NBLK):
                if sb == 0:
                    nc.vector.tensor_copy(ycols(sb, 0, B), xmb_slice(sb))
                else:
                    q_fin(sb, pp_cur)
                if sb + 1 < NBLK:
                    pp = psum_e.tile([128, B * K], fp32, tag="pp")
                    if sb >= 1:  # overlaps this block's steps
                        p_early(sb + 1, pp)
                else:
                    pp = None
                steps(sb)
                if sb + 1 < NBLK:
                    transpose_block(sb)
                    p_final(sb + 1, pp, had_early=sb >= 1)
                pp_cur = pp

            def big_out(dram, tile):
                nc.sync.dma_start(
                    dram[:].rearrange("(k p) c -> p k c", p=128),
                    tile[:].rearrange("p (k c) -> p k c", c=C),
                )

            big_out(yt, Y)
            big_out(mot, MIX)

    nc.compile()
    return nc


def get_nc(n_chunks=NPIX // 128):
    if n_chunks not in _CACHE:
        _CACHE[n_chunks] = _build(n_chunks)
    return _CACHE[n_chunks]


def make_core_inputs(x, mix, W, b):
    """Host-side layout prep. Returns list of per-core input dicts."""
    Wm = (W * np.tril(np.ones((C - 1, C), np.float32))).astype(np.float32)
    wt = np.zeros((C, C), np.float32)
    wt[:, 1:] = Wm.T  # wt[c, i] = Wm[i-1, c]
    # in-block triangle, negated, with +1 on the diagonal: the scan over
    # [y_0..y_{i-1}, q_i] then yields t_i = q_i - sum_j w_ij y_j directly
    wtri = np.zeros((NBLK, B, B), np.float32)
    for sb in range(NBLK):
        for i in range(1, B):
            ch = sb * B + i
            wtri[sb, i, :i] = -Wm[ch - 1, sb * B : sb * B + i]
            wtri[sb, i, i] = 1.0
    wtri = wtri.reshape(1, -1)
    bpad = np.zeros((C,), np.float32)
    bpad[1:] = b
    ident = np.eye(128, dtype=np.float32)

    in_maps = []
    for n in range(N):
        xtn = np.ascontiguousarray(x[n].reshape(C, NPIX).T)
        mixn = np.ascontiguousarray(
            (mix[n] + bpad[:, None, None]).reshape(C, NPIX).T
        )
        in_maps.append(
            {"xt": xtn, "mixt": mixn, "wt": wt, "wtri": wtri, "ident": ident}
        )
    return in_maps


def kernel(x, mix, W, b):
    from concourse.bass_utils import run_bass_kernel_spmd

    x = np.asarray(x, np.float32)
    mix = np.asarray(mix, np.float32)
    W = np.asarray(W, np.float32)
    b = np.asarray(b, np.float32)

    nc = get_nc()
    in_maps = make_core_inputs(x, mix, W, b)
    res = run_bass_kernel_spmd(nc, in_maps, list(range(N)))

    ybar = np.empty((N, C, H, Wd), np.float32)
    mix_out = np.empty((N, C, H, Wd), np.float32)
    for n in range(N):
        ybar[n] = res.results[n]["yt"].T.reshape(C, H, Wd)
        mix_out[n] = res.results[n]["mot"].T.reshape(C, H, Wd)
    mix_out[:, 0] = mix[:, 0]  # reference passes mix ch0 through exactly
    return ybar, mix_out



# revision 2
# speedup vs baseline: 1.0740x; 1.0740x over previous
"""Trainium2 Bass kernel for ChannelDepsModule (sequential channel recurrence).

Math (per pixel, fp32):
    m_0 = mix_0 ; ybar_0 = round(x_0 - m_0) + m_0
    for i in 1..191:
        m_i = sum_{c<i} Wm[i-1,c] * ybar_c + b[i-1] + mix_i
        ybar_i = round(x_i - m_i) + m_i
    outputs: ybar, mix_out (= m)

Device strategy (per core, one batch image, 4096 pixels):
  - pixels on SBUF partitions ([128] x 32 chunks), channels on the free dim
  - channels in 6 blocks of 32:
      * cross-block mix contributions P via TensorE matmuls
        (stationary ybar in channel-partition layout x Wm^T slice), with
        pixel-partition PSUM output; q = x - mix - b - P is written into the
        block's ybar columns ahead of time
      * in-block recurrence: one fused DVE scan per channel computes
        t_i = q_i - sum_j w_ij y_j directly (weights negated, +1 planted on
        the diagonal so the prefilled q column enters the dot); the scan's
        OUTPUT access pattern is a stride-0 broadcast of column i of a
        per-block T tile, so the final element (t_i) persists there; then
        one fused DVE op assembles y_i = round(t_i) + (x_i - t_i) using the
        1.5*2^23 magic constant (IEEE RNE == jnp.round)
      * mix_out block = X - T in ONE DVE op per block (no GpSimd at all)
      * finished ybar columns are TensorE-transposed in 4-chunk groups,
        interleaved with the next block's final P matmuls per group
  - inputs DMA'd in 3 channel-groups so block 0 starts early; outputs
    DMA'd out per block so the tail is short
  - b is folded into mix on the host; mix_out channel 0 restored on host
"""

import sys

import numpy as np

if "/opt/trn_rl_repo" not in sys.path:
    sys.path.insert(0, "/opt/trn_rl_repo")

N, C, H, Wd = 8, 192, 64, 64
NPIX = H * Wd          # 4096 pixels per core
B = 32                 # channel block size
NBLK = C // B          # 6
ROUND_C = 1.5 * 2.0**23  # fp32 add of this rounds to nearest-even integer

_CACHE = {}
_DVE_OPS = {}


def _register_dve_ops():
    """Define + register the two fused DVE ops (idempotent)."""
    if _DVE_OPS:
        return _DVE_OPS
    import concourse.dve_ops as dops
    import concourse.dve_spec as ds
    from concourse.dve_spec import AluOp, Spec, Src0, Src1
    from concourse.dve_ops import CUSTOM_DVE_SPECS, OPS, DveOp
    from concourse.dve_uop import DveOpSpec

    # The stock segmented-scan machinery only implements the page-counter
    # mode; add the documented per-page *reset* behavior for scans marked
    # with `_page_reset`: at each SUB_DIM_DONE the STEP state computes
    # d <- op(init, expr) instead of op(CURR, expr).
    if not getattr(ds, "_page_reset_patched", False):
        _orig = ds._scan_overrides

        def _patched(scans, node_stage):
            seed, step = _orig(scans, node_stage)
            for sc in scans:
                if getattr(sc, "_page_reset", False):
                    d = node_stage[sc]
                    step[d] = ds._Stage(sc.op, ds._scan_init(sc), sc.expr)
            return seed, step

        ds._scan_overrides = _patched
        ds._page_reset_patched = True

    def _chaindot_ref(in0, in1, s0, s1, imm2):
        p = in0.shape[0]
        inner = in0.shape[-1]
        a = in0.reshape(p, -1, inner).astype(np.float32)
        bb = in1.reshape(p, -1, inner).astype(np.float32)
        return np.cumsum(a * bb, axis=-1, dtype=np.float32).reshape(in0.shape)

    sc = ds.scan(AluOp.ADD, Src0 * Src1)
    object.__setattr__(sc, "_page_reset", True)
    spec_cd = Spec(body=sc, reference=_chaindot_ref)

    def _quanty_ref(in0, in1, s0, s1, imm2):
        c = np.float32(s0)
        t = in0.astype(np.float32)
        return ((t + c) - c) + (in1.astype(np.float32) - t)

    spec_qy = Spec(
        body=((Src0 + ds.C0) - ds.C0) + (Src1 - Src0), reference=_quanty_ref
    )

    def _mk(name, spec, subdim):
        if any(o.name == name for o in OPS):
            op = next(o for o in OPS if o.name == name)
        else:
            shas = {}
            for ver in ("v3", "v4"):
                shas[ver] = DveOpSpec(
                    name=name, uops=ds.lower(spec, ver=ver)
                ).sha(ver)
            op = DveOp(name, spec, subdim=subdim, uops_sha=shas)
            OPS.append(op)
            CUSTOM_DVE_SPECS[name] = spec
            dops._SUB_OPCODE_FOR_NAME[name] = dops._CUSTOM_DVE_ROW_BASE + len(OPS) - 1
        return op

    _DVE_OPS["chaindot"] = _mk("CHAINDOT_SEQ_ANT", spec_cd, subdim=True)
    _DVE_OPS["quanty"] = _mk("QUANTY_ANT", spec_qy, subdim=False)
    return _DVE_OPS


def _build(n_chunks):
    """Build + compile the per-core Bass module. n_chunks pixel chunks of 128."""
    import concourse.bacc as bacc
    import concourse.mybir as mybir
    from concourse.tile import TileContext

    ops = _register_dve_ops()
    npix = n_chunks * 128
    fp32 = mybir.dt.float32

    nc = bacc.Bacc(None, target_bir_lowering=False)

    xt = nc.dram_tensor("xt", [npix, C], fp32, kind="ExternalInput")
    mixt = nc.dram_tensor("mixt", [npix, C], fp32, kind="ExternalInput")
    wt = nc.dram_tensor("wt", [C, C], fp32, kind="ExternalInput")
    wtrib = nc.dram_tensor("wtrib", [128, NBLK * B * B], fp32, kind="ExternalInput")
    ident = nc.dram_tensor("ident", [128, 128], fp32, kind="ExternalInput")
    yt = nc.dram_tensor("yt", [npix, C], fp32, kind="ExternalOutput")
    mot = nc.dram_tensor("mot", [npix, C], fp32, kind="ExternalOutput")

    K = n_chunks  # pixel chunks
    GRP = C // 3  # input DMA channel-group width (64)

    with TileContext(nc) as tc:
        with (
            tc.tile_pool(name="big", bufs=1) as big,
            tc.tile_pool(name="small", bufs=1) as small,
            tc.tile_pool(name="tp", bufs=2) as tpool,
            tc.tile_pool(name="psum_e", bufs=2, space="PSUM") as psum_e,
            tc.tile_pool(name="psumt", bufs=2, space="PSUM") as psumt,
        ):
            # pixel-partition tiles, free layout = k*192 + c
            X = big.tile([128, K * C], fp32, tag="X")
            MIX = big.tile([128, K * C], fp32, tag="MIX")  # becomes mix_out
            XMB = big.tile([128, K * C], fp32, tag="XMB")
            Y = big.tile([128, K * C], fp32, tag="Y")
            # channel-partition decoded ybar: chans 0-127 / 128-159
            ysb_lo = big.tile([128, npix], fp32, tag="ysb_lo")
            ysb_hi = big.tile([32, npix], fp32, tag="ysb_hi")

            wt_lo = small.tile([128, C], fp32, tag="wt_lo")
            wt_hi = small.tile([64, C], fp32, tag="wt_hi")
            wtri_b = small.tile([128, NBLK * B * B], fp32, tag="wtri_b")
            id_t = small.tile([128, 128], fp32, tag="ident")

            def reKC(tile):
                return tile[:].rearrange("p (k c) -> p k c", c=C)

            def grp_in(tile, dram, g):
                nc.sync.dma_start(
                    reKC(tile)[:, :, g * GRP : (g + 1) * GRP],
                    dram[:].rearrange("(k p) c -> p k c", p=128)[
                        :, :, g * GRP : (g + 1) * GRP
                    ],
                )

            # input DMAs, ordered so block 0 can start early
            grp_in(X, xt, 0)
            grp_in(MIX, mixt, 0)
            nc.sync.dma_start(wtri_b[:], wtrib[:])
            grp_in(X, xt, 1)
            grp_in(MIX, mixt, 1)
            grp_in(X, xt, 2)
            grp_in(MIX, mixt, 2)
            nc.sync.dma_start(wt_lo[:], wt[0:128, :])
            nc.sync.dma_start(wt_hi[:], wt[128:C, :])
            nc.sync.dma_start(id_t[:], ident[:])

            def xmb_grp(g):
                sl = slice(g * GRP, (g + 1) * GRP)
                nc.vector.tensor_sub(
                    reKC(XMB)[:, :, sl], reKC(X)[:, :, sl], reKC(MIX)[:, :, sl]
                )

            xmb_grp(0)

            def col(tile, ch):  # strided [128, K] view of channel ch
                return reKC(tile)[:, :, ch]

            def ycols(sb, j0, j1):  # [128, K, j1-j0] view of block sb's cols
                return reKC(Y)[:, :, sb * B + j0 : sb * B + j1]

            def xmb_slice(sb):
                return reKC(XMB)[:, :, sb * B : (sb + 1) * B]

            # block 0 prefill: q = x - mixb (no P)
            nc.vector.tensor_copy(ycols(0, 0, B), xmb_slice(0))

            # PSUM accumulation-group bookkeeping: start=True marks the whole
            # 2KB bank pending-zero, so the first matmul touching each bank
            # opens the group (later writes to untouched bytes overwrite,
            # rewrites accumulate) and the last one per bank closes it.
            BANK_CHUNKS = 512 // B  # chunk-slices per PSUM bank

            def p_early(sb, pp, k0, k1):
                """Early P for block sb, chunks [k0,k1): chans [0, 32(sb-1))."""
                kdec = (sb - 1) * B
                for k in range(k0, k1):
                    nc.tensor.matmul(
                        pp[:, k * B : (k + 1) * B],
                        ysb_lo[0:kdec, k * 128 : (k + 1) * 128],
                        wt_lo[0:kdec, sb * B : (sb + 1) * B],
                        start=(k % BANK_CHUNKS == 0),
                        stop=False,
                    )

            def p_final_chunk(sb, pp, had_early, k):
                """Final P seg for block sb, one chunk: block sb-1's 32 chans."""
                r0 = (sb - 1) * B
                ys, wtile, rr = (
                    (ysb_lo, wt_lo, r0) if r0 < 128 else (ysb_hi, wt_hi, r0 - 128)
                )
                tp = (rr, 0) if rr not in (0, 32, 64) else None
                nc.tensor.matmul(
                    pp[:, k * B : (k + 1) * B],
                    ys[rr : rr + B, k * 128 : (k + 1) * 128],
                    wtile[rr : rr + B, sb * B : (sb + 1) * B],
                    start=(not had_early) and (k % BANK_CHUNKS == 0),
                    stop=(k % BANK_CHUNKS == BANK_CHUNKS - 1) or (k == K - 1),
                    tile_position=tp,
                )

            def q_fin(sb, pp):
                """Block sb's Y cols = XMB - PP."""
                nc.vector.tensor_sub(
                    ycols(sb, 0, B),
                    xmb_slice(sb),
                    pp[:].rearrange("p (k c) -> p k c", c=B),
                )

            def boundary(sb, pp, had_early):
                """Transpose block sb's Y cols into ysb (chan-part layout),
                interleaved per 4-chunk group with block sb+1's final P."""
                base = sb * B
                if base < 128:
                    dst, dr0 = ysb_lo, base
                else:
                    dst, dr0 = ysb_hi, base - 128
                for g in range(0, K, 4):
                    gn = min(4, K - g)
                    pt = psumt.tile([B, 512], fp32, tag="pt")
                    for t_i in range(gn):
                        k = g + t_i
                        nc.tensor.transpose(
                            pt[:, t_i * 128 : (t_i + 1) * 128],
                            Y[:, k * C + base : k * C + base + B],
                            id_t[:],
                        )
                    nc.scalar.copy(
                        dst[dr0 : dr0 + B, g * 128 : g * 128 + gn * 128],
                        pt[:, 0 : gn * 128],
                    )
                    for t_i in range(gn):
                        p_final_chunk(sb + 1, pp, had_early, g + t_i)

            def steps(sb, T):
                base = sb * B
                tv = T[:].rearrange("p (k c) -> p k c", c=B)
                for i in range(B):
                    ch = base + i
                    woff = sb * B * B + i * B
                    wrow = (
                        wtri_b[:, woff : woff + i + 1]
                        .unsqueeze(1)
                        .broadcast_to([128, K, i + 1])
                    )
                    tcol = tv[:, :, i].unsqueeze(2).broadcast_to([128, K, i + 1])
                    # t_i = q_i - sum_j w_ij y_j ; lands in T col i (stride-0
                    # out: every scan position writes the same column, the
                    # final value is the page's full dot)
                    nc.vector._custom_dve(
                        ops["chaindot"], out=tcol, in0=ycols(sb, 0, i + 1),
                        in1=wrow,
                    )
                    # y = round(t) + (x - t)
                    nc.vector._custom_dve(
                        ops["quanty"], out=col(Y, ch), in0=tv[:, :, i],
                        in1=col(X, ch), s0=ROUND_C,
                    )
                    # prologue compute for later groups, tucked between scans
                    if sb == 0 and i == 8:
                        xmb_grp(1)
                    elif sb == 0 and i == 20:
                        xmb_grp(2)
                    # early cross-block P for block sb+1 (older blocks only)
                    if 1 <= sb <= NBLK - 2:
                        if i == 4:
                            p_early(sb + 1, steps.pp_next, 0, K // 2)
                        elif i == 16:
                            p_early(sb + 1, steps.pp_next, K // 2, K)

            def m_block(sb, T):
                """mix_out block sb = X - T (all 32 t-columns)."""
                tv = T[:].rearrange("p (k c) -> p k c", c=B)
                nc.vector.tensor_sub(
                    reKC(MIX)[:, :, sb * B : (sb + 1) * B],
                    reKC(X)[:, :, sb * B : (sb + 1) * B],
                    tv,
                )

            def dma_out_block(sb):
                sl = slice(sb * B, (sb + 1) * B)
                nc.sync.dma_start(
                    yt[:].rearrange("(k p) c -> p k c", p=128)[:, :, sl],
                    reKC(Y)[:, :, sl],
                )
                nc.sync.dma_start(
                    mot[:].rearrange("(k p) c -> p k c", p=128)[:, :, sl],
                    reKC(MIX)[:, :, sl],
                )

            # ---------------- schedule ----------------
            for sb in range(NBLK):
                T = tpool.tile([128, K * B], fp32, tag="T")
                if 1 <= sb <= NBLK - 2:
                    # pp for block sb+1, filled by p_early during steps(sb)
                    steps.pp_next = psum_e.tile([128, B * K], fp32, tag="pp")
                steps(sb, T)
                m_block(sb, T)
                dma_out_block(sb)
                if sb + 1 < NBLK:
                    if sb == 0:
                        pp = psum_e.tile([128, B * K], fp32, tag="pp")
                        had_early = False
                    else:
                        pp = steps.pp_next
                        had_early = True
                    boundary(sb, pp, had_early)
                    q_fin(sb + 1, pp)

    nc.compile()
    return nc


def get_nc(n_chunks=NPIX // 128):
    if n_chunks not in _CACHE:
        _CACHE[n_chunks] = _build(n_chunks)
    return _CACHE[n_chunks]


def make_core_inputs(x, mix, W, b):
    """Host-side layout prep. Returns list of per-core input dicts."""
    Wm = (W * np.tril(np.ones((C - 1, C), np.float32))).astype(np.float32)
    wt = np.zeros((C, C), np.float32)
    wt[:, 1:] = Wm.T  # wt[c, i] = Wm[i-1, c]
    # in-block triangle, negated, with +1 on the diagonal: the scan over
    # [y_0..y_{i-1}, q_i] then yields t_i = q_i - sum_j w_ij y_j directly
    wtri = np.zeros((NBLK, B, B), np.float32)
    for sb in range(NBLK):
        wtri[sb, 0, 0] = 1.0
        for i in range(1, B):
            ch = sb * B + i
            wtri[sb, i, :i] = -Wm[ch - 1, sb * B : sb * B + i]
            wtri[sb, i, i] = 1.0
    wtrib = np.broadcast_to(
        wtri.reshape(1, -1), (128, NBLK * B * B)
    ).copy()
    bpad = np.zeros((C,), np.float32)
    bpad[1:] = b
    ident = np.eye(128, dtype=np.float32)

    in_maps = []
    for n in range(N):
        xtn = np.ascontiguousarray(x[n].reshape(C, NPIX).T)
        mixn = np.ascontiguousarray(
            (mix[n] + bpad[:, None, None]).reshape(C, NPIX).T
        )
        in_maps.append(
            {"xt": xtn, "mixt": mixn, "wt": wt, "wtrib": wtrib, "ident": ident}
        )
    return in_maps


def kernel(x, mix, W, b):
    from concourse.bass_utils import run_bass_kernel_spmd

    x = np.asarray(x, np.float32)
    mix = np.asarray(mix, np.float32)
    W = np.asarray(W, np.float32)
    b = np.asarray(b, np.float32)

    nc = get_nc()
    in_maps = make_core_inputs(x, mix, W, b)
    res = run_bass_kernel_spmd(nc, in_maps, list(range(N)))

    ybar = np.empty((N, C, H, Wd), np.float32)
    mix_out = np.empty((N, C, H, Wd), np.float32)
    for n in range(N):
        ybar[n] = res.results[n]["yt"].T.reshape(C, H, Wd)
        mix_out[n] = res.results[n]["mot"].T.reshape(C, H, Wd)
    mix_out[:, 0] = mix[:, 0]  # reference passes mix ch0 through exactly
    return ybar, mix_out


# revision 11
# speedup vs baseline: 1.1731x; 1.0922x over previous
"""Trainium2 Bass kernel for ChannelDepsModule (sequential channel recurrence).

Math (per pixel, fp32):
    m_0 = mix_0 ; ybar_0 = round(x_0 - m_0) + m_0
    for i in 1..191:
        m_i = sum_{c<i} Wm[i-1,c] * ybar_c + b[i-1] + mix_i
        ybar_i = round(x_i - m_i) + m_i
    outputs: ybar, mix_out (= m)

Device strategy (per core, one batch image, 4096 pixels):
  - pixels on SBUF partitions ([128] x 32 chunks), channels on the free dim
  - channels in 6 blocks of 32; pixels in 2 halves of 16 chunks that are
    software-pipelined: while the DVE runs half H1's sequential scans, the
    TensorE transposes half H0's finished block and computes its next-block
    cross-P, so the DVE never waits at a block boundary
  - per channel, one fused DVE scan computes t_i = q_i - sum_j w_ij y_j
    (weights negated, +1 on the diagonal; q = x - mix - b - P prefilled)
    writing t_i into a per-block T tile via a stride-0 broadcast output;
    a second fused DVE op assembles y_i = round(t_i) + (x_i - t_i)
  - mix_out block = X - T in one GpSimd op per (block, half)
  - cross-block P via TensorE matmuls (stationary = transposed ybar slice,
    moving = Wm^T slice), accumulated per (block, half) in one PSUM bank
  - inputs DMA'd in 3 channel-groups so block 0 starts early; outputs
    DMA'd out per block so the tail is short
  - b is folded into mix on the host; mix_out channel 0 restored on host
"""

import sys

import numpy as np

if "/opt/trn_rl_repo" not in sys.path:
    sys.path.insert(0, "/opt/trn_rl_repo")

N, C, H, Wd = 8, 192, 64, 64
NPIX = H * Wd          # 4096 pixels per core
B = 32                 # channel block size
NBLK = C // B          # 6
ROUND_C = 1.5 * 2.0**23  # fp32 add of this rounds to nearest-even integer

_CACHE = {}
_DVE_OPS = {}


def _register_dve_ops():
    """Define + register the two fused DVE ops (idempotent)."""
    if _DVE_OPS:
        return _DVE_OPS
    import concourse.dve_ops as dops
    import concourse.dve_spec as ds
    from concourse.dve_spec import AluOp, Spec, Src0, Src1
    from concourse.dve_ops import CUSTOM_DVE_SPECS, OPS, DveOp
    from concourse.dve_uop import DveOpSpec

    # The stock segmented-scan machinery only implements the page-counter
    # mode; add the documented per-page *reset* behavior for scans marked
    # with `_page_reset`: at each SUB_DIM_DONE the STEP state computes
    # d <- op(init, expr) instead of op(CURR, expr).
    if not getattr(ds, "_page_reset_patched", False):
        _orig = ds._scan_overrides

        def _patched(scans, node_stage):
            seed, step = _orig(scans, node_stage)
            for sc in scans:
                if getattr(sc, "_page_reset", False):
                    d = node_stage[sc]
                    step[d] = ds._Stage(sc.op, ds._scan_init(sc), sc.expr)
            return seed, step

        ds._scan_overrides = _patched
        ds._page_reset_patched = True

    def _chaindot_ref(in0, in1, s0, s1, imm2):
        p = in0.shape[0]
        inner = in0.shape[-1]
        a = in0.reshape(p, -1, inner).astype(np.float32)
        bb = in1.reshape(p, -1, inner).astype(np.float32)
        return np.cumsum(a * bb, axis=-1, dtype=np.float32).reshape(in0.shape)

    sc = ds.scan(AluOp.ADD, Src0 * Src1)
    object.__setattr__(sc, "_page_reset", True)
    spec_cd = Spec(body=sc, reference=_chaindot_ref)

    def _quanty_ref(in0, in1, s0, s1, imm2):
        c = np.float32(s0)
        t = in0.astype(np.float32)
        return ((t + c) - c) + (in1.astype(np.float32) - t)

    spec_qy = Spec(
        body=((Src0 + ds.C0) - ds.C0) + (Src1 - Src0), reference=_quanty_ref
    )

    def _mk(name, spec, subdim):
        if any(o.name == name for o in OPS):
            op = next(o for o in OPS if o.name == name)
        else:
            shas = {}
            for ver in ("v3", "v4"):
                shas[ver] = DveOpSpec(
                    name=name, uops=ds.lower(spec, ver=ver)
                ).sha(ver)
            op = DveOp(name, spec, subdim=subdim, uops_sha=shas)
            OPS.append(op)
            CUSTOM_DVE_SPECS[name] = spec
            dops._SUB_OPCODE_FOR_NAME[name] = dops._CUSTOM_DVE_ROW_BASE + len(OPS) - 1
        return op

    _DVE_OPS["chaindot"] = _mk("CHAINDOT_SEQ_ANT", spec_cd, subdim=True)
    _DVE_OPS["quanty"] = _mk("QUANTY_ANT", spec_qy, subdim=False)
    return _DVE_OPS


def _build(n_chunks):
    """Build + compile the per-core Bass module. n_chunks pixel chunks of 128."""
    import concourse.bacc as bacc
    import concourse.mybir as mybir
    from concourse.tile import TileContext

    ops = _register_dve_ops()
    npix = n_chunks * 128
    fp32 = mybir.dt.float32

    nc = bacc.Bacc(None, target_bir_lowering=False)

    xt = nc.dram_tensor("xt", [npix, C], fp32, kind="ExternalInput")
    mixt = nc.dram_tensor("mixt", [npix, C], fp32, kind="ExternalInput")
    wt = nc.dram_tensor("wt", [C, C], fp32, kind="ExternalInput")
    wtrib = nc.dram_tensor("wtrib", [128, NBLK * B * B], fp32, kind="ExternalInput")
    ident = nc.dram_tensor("ident", [128, 128], fp32, kind="ExternalInput")
    yt = nc.dram_tensor("yt", [npix, C], fp32, kind="ExternalOutput")
    mot = nc.dram_tensor("mot", [npix, C], fp32, kind="ExternalOutput")

    K = n_chunks   # pixel chunks
    KH = K // 2    # chunks per pipeline half
    GRP = C // 3   # input DMA channel-group width (64)

    with TileContext(nc) as tc:
        with (
            tc.tile_pool(name="big", bufs=1) as big,
            tc.tile_pool(name="small", bufs=1) as small,
            tc.tile_pool(name="tp", bufs=2) as tpool,
            tc.tile_pool(name="psum_e", bufs=4, space="PSUM") as psum_e,
            tc.tile_pool(name="psumt", bufs=2, space="PSUM") as psumt,
        ):
            # pixel-partition tiles, free layout = k*192 + c
            X = big.tile([128, K * C], fp32, tag="X")
            MIX = big.tile([128, K * C], fp32, tag="MIX")  # becomes mix_out
            XMB = big.tile([128, K * C], fp32, tag="XMB")
            Y = big.tile([128, K * C], fp32, tag="Y")
            # channel-partition decoded ybar: chans 0-127 / 128-159
            ysb_lo = big.tile([128, npix], fp32, tag="ysb_lo")
            ysb_hi = big.tile([32, npix], fp32, tag="ysb_hi")

            wt_lo = small.tile([128, C], fp32, tag="wt_lo")
            wt_hi = small.tile([64, C], fp32, tag="wt_hi")
            wtri_b = small.tile([128, NBLK * B * B], fp32, tag="wtri_b")
            id_t = small.tile([128, 128], fp32, tag="ident")

            def reKC(tile):
                return tile[:].rearrange("p (k c) -> p k c", c=C)

            def grp_in(tile, dram, g):
                nc.sync.dma_start(
                    reKC(tile)[:, :, g * GRP : (g + 1) * GRP],
                    dram[:].rearrange("(k p) c -> p k c", p=128)[
                        :, :, g * GRP : (g + 1) * GRP
                    ],
                )

            # input DMAs, ordered so block 0 can start early
            grp_in(X, xt, 0)
            grp_in(MIX, mixt, 0)
            nc.sync.dma_start(wtri_b[:], wtrib[:])
            grp_in(X, xt, 1)
            grp_in(MIX, mixt, 1)
            grp_in(X, xt, 2)
            grp_in(MIX, mixt, 2)
            nc.sync.dma_start(wt_lo[:], wt[0:128, :])
            nc.sync.dma_start(wt_hi[:], wt[128:C, :])
            nc.sync.dma_start(id_t[:], ident[:])

            def xmb_grp(g, eng):
                sl = slice(g * GRP, (g + 1) * GRP)
                eng.tensor_sub(
                    reKC(XMB)[:, :, sl], reKC(X)[:, :, sl], reKC(MIX)[:, :, sl]
                )

            xmb_grp(0, nc.vector)

            def khs(h):  # chunk-half slice
                return slice(h * KH, (h + 1) * KH)

            def col(tile, ch, h):  # strided [128, KH] view of channel ch
                return reKC(tile)[:, khs(h), ch]

            def ycols(sb, j0, j1, h):
                return reKC(Y)[:, khs(h), sb * B + j0 : sb * B + j1]

            def xmb_slice(sb, h):
                return reKC(XMB)[:, khs(h), sb * B : (sb + 1) * B]

            # block 0 prefill: q = x - mixb (no P)
            nc.vector.tensor_copy(
                reKC(Y)[:, :, 0:B], reKC(XMB)[:, :, 0:B]
            )

            # Each pp tile is one PSUM bank: [128, B*KH] covering one
            # (target-block, pixel-half). start on its first chunk,
            # stop on its last.
            def p_early(sb, pp, h):
                """Early P for block sb, half h: chans [0, 32(sb-1))."""
                kdec = (sb - 1) * B
                for kl in range(KH):
                    k = h * KH + kl
                    nc.tensor.matmul(
                        pp[:, kl * B : (kl + 1) * B],
                        ysb_lo[0:kdec, k * 128 : (k + 1) * 128],
                        wt_lo[0:kdec, sb * B : (sb + 1) * B],
                        start=(kl == 0),
                        stop=False,
                    )

            def p_final_chunk(sb, pp, had_early, k):
                """Final P seg for block sb, one chunk: block sb-1's chans."""
                kl = k % KH
                r0 = (sb - 1) * B
                ys, wtile, rr = (
                    (ysb_lo, wt_lo, r0) if r0 < 128 else (ysb_hi, wt_hi, r0 - 128)
                )
                tp = (rr, 0) if rr not in (0, 32, 64) else None
                nc.tensor.matmul(
                    pp[:, kl * B : (kl + 1) * B],
                    ys[rr : rr + B, k * 128 : (k + 1) * 128],
                    wtile[rr : rr + B, sb * B : (sb + 1) * B],
                    start=(not had_early) and (kl == 0),
                    stop=(kl == KH - 1),
                    tile_position=tp,
                )

            def q_fin(sb, pp, h):
                """Block sb's Y cols (half h) = XMB - PP."""
                nc.vector.tensor_sub(
                    ycols(sb, 0, B, h),
                    xmb_slice(sb, h),
                    pp[:].rearrange("p (k c) -> p k c", c=B),
                )

            def tr_pf(sb, pp, had_early, h):
                """Transpose block sb's half-h Y cols into ysb, interleaved
                per 4-chunk group with block sb+1's final P for half h."""
                base = sb * B
                if base < 128:
                    dst, dr0 = ysb_lo, base
                else:
                    dst, dr0 = ysb_hi, base - 128
                for g in range(0, KH, 4):
                    pt = psumt.tile([B, 512], fp32, tag="pt")
                    for t_i in range(4):
                        k = h * KH + g + t_i
                        nc.tensor.transpose(
                            pt[:, t_i * 128 : (t_i + 1) * 128],
                            Y[:, k * C + base : k * C + base + B],
                            id_t[:],
                        )
                    k0 = h * KH + g
                    nc.scalar.copy(
                        dst[dr0 : dr0 + B, k0 * 128 : (k0 + 4) * 128],
                        pt[:],
                    )
                    for t_i in range(4):
                        p_final_chunk(sb + 1, pp, had_early, k0 + t_i)

            def steps(sb, T, h):
                base = sb * B
                tv = T[:].rearrange("p (k c) -> p k c", c=B)
                for i in range(B):
                    ch = base + i
                    woff = sb * B * B + i * B
                    wrow = (
                        wtri_b[:, woff : woff + i + 1]
                        .unsqueeze(1)
                        .broadcast_to([128, KH, i + 1])
                    )
                    tcol = tv[:, :, i].unsqueeze(2).broadcast_to([128, KH, i + 1])
                    # t_i = q_i - sum_j w_ij y_j ; lands in T col i (stride-0
                    # out: every scan position writes the same column, the
                    # final value is the page's full dot)
                    nc.vector._custom_dve(
                        ops["chaindot"], out=tcol, in0=ycols(sb, 0, i + 1, h),
                        in1=wrow,
                    )
                    # y = round(t) + (x - t)
                    nc.vector._custom_dve(
                        ops["quanty"], out=col(Y, ch, h), in0=tv[:, :, i],
                        in1=col(X, ch, h), s0=ROUND_C,
                    )
                    # prologue compute for later groups, on idle GpSimd
                    if sb == 0 and h == 0 and i == 8:
                        xmb_grp(1, nc.gpsimd)
                    elif sb == 0 and h == 0 and i == 24:
                        xmb_grp(2, nc.gpsimd)

            def m_block(sb, T, h):
                """mix_out block sb half h = X - T, on GpSimd (off DVE)."""
                tv = T[:].rearrange("p (k c) -> p k c", c=B)
                nc.gpsimd.tensor_sub(
                    reKC(MIX)[:, khs(h), sb * B : (sb + 1) * B],
                    reKC(X)[:, khs(h), sb * B : (sb + 1) * B],
                    tv,
                )

            def dma_out_block(sb):
                sl = slice(sb * B, (sb + 1) * B)
                nc.sync.dma_start(
                    yt[:].rearrange("(k p) c -> p k c", p=128)[:, :, sl],
                    reKC(Y)[:, :, sl],
                )
                nc.sync.dma_start(
                    mot[:].rearrange("(k p) c -> p k c", p=128)[:, :, sl],
                    reKC(MIX)[:, :, sl],
                )

            # ---------------- software-pipelined schedule ----------------
            # DVE:    s(0,H0) s(0,H1) | qf(1,H0) s(1,H0) qf(1,H1) s(1,H1) | ..
            # TensorE runs each half's transpose + final-P while the DVE
            # scans the other half; the two p_early halves for block sb+2
            # are spread one per window to balance TensorE load.
            pp = {}

            for sb in range(NBLK):
                for h in (0, 1):
                    T = tpool.tile([128, KH * B], fp32, tag=f"T{h}")
                    if sb > 0:
                        q_fin(sb, pp.pop((sb, h)), h)
                    steps(sb, T, h)
                    m_block(sb, T, h)
                    if sb + 1 < NBLK:
                        if sb == 0:
                            pp[(1, h)] = psum_e.tile(
                                [128, B * KH], fp32, tag="pp", name=f"pp1{h}"
                            )
                        tr_pf(sb, pp[(sb + 1, h)], had_early=(sb > 0), h=h)
                    if h == 0 and 2 <= sb + 1 < NBLK:
                        # second-half early P for block sb+1 (emitted after
                        # tr/pf(sb, H0); runs during scans(sb, H1))
                        p_early(sb + 1, pp[(sb + 1, 1)], 1)
                    if h == 1 and sb + 2 < NBLK:
                        pp[(sb + 2, 0)] = psum_e.tile(
                            [128, B * KH], fp32, tag="pp", name=f"pp{sb+2}0"
                        )
                        pp[(sb + 2, 1)] = psum_e.tile(
                            [128, B * KH], fp32, tag="pp", name=f"pp{sb+2}1"
                        )
                        # first-half early P for block sb+2 (runs during
                        # scans(sb+1, H0))
                        p_early(sb + 2, pp[(sb + 2, 0)], 0)
                dma_out_block(sb)

    nc.compile()
    return nc


def get_nc(n_chunks=NPIX // 128):
    if n_chunks not in _CACHE:
        _CACHE[n_chunks] = _build(n_chunks)
    return _CACHE[n_chunks]


def make_core_inputs(x, mix, W, b):
    """Host-side layout prep. Returns list of per-core input dicts."""
    Wm = (W * np.tril(np.ones((C - 1, C), np.float32))).astype(np.float32)
    wt = np.zeros((C, C), np.float32)
    wt[:, 1:] = Wm.T  # wt[c, i] = Wm[i-1, c]
    # in-block triangle, negated, with +1 on the diagonal: the scan over
    # [y_0..y_{i-1}, q_i] then yields t_i = q_i - sum_j w_ij y_j directly
    wtri = np.zeros((NBLK, B, B), np.float32)
    for sb in range(NBLK):
        wtri[sb, 0, 0] = 1.0
        for i in range(1, B):
            ch = sb * B + i
            wtri[sb, i, :i] = -Wm[ch - 1, sb * B : sb * B + i]
            wtri[sb, i, i] = 1.0
    wtrib = np.broadcast_to(
        wtri.reshape(1, -1), (128, NBLK * B * B)
    ).copy()
    bpad = np.zeros((C,), np.float32)
    bpad[1:] = b
    ident = np.eye(128, dtype=np.float32)

    in_maps = []
    for n in range(N):
        xtn = np.ascontiguousarray(x[n].reshape(C, NPIX).T)
        mixn = np.ascontiguousarray(
            (mix[n] + bpad[:, None, None]).reshape(C, NPIX).T
        )
        in_maps.append(
            {"xt": xtn, "mixt": mixn, "wt": wt, "wtrib": wtrib, "ident": ident}
        )
    return in_maps


def kernel(x, mix, W, b):
    from concourse.bass_utils import run_bass_kernel_spmd

    x = np.asarray(x, np.float32)
    mix = np.asarray(mix, np.float32)
    W = np.asarray(W, np.float32)
    b = np.asarray(b, np.float32)

    nc = get_nc()
    in_maps = make_core_inputs(x, mix, W, b)
    res = run_bass_kernel_spmd(nc, in_maps, list(range(N)))

    ybar = np.empty((N, C, H, Wd), np.float32)
    mix_out = np.empty((N, C, H, Wd), np.float32)
    for n in range(N):
        ybar[n] = res.results[n]["yt"].T.reshape(C, H, Wd)
        mix_out[n] = res.results[n]["mot"].T.reshape(C, H, Wd)
    mix_out[:, 0] = mix[:, 0]  # reference passes mix ch0 through exactly
    return ybar, mix_out
